# revision 27
# baseline (speedup 1.0000x reference)
"""Trainium2 Bass kernel for nn_Bottleneck_5669356834470 (ResNet bottleneck
with an involution middle layer).

Sharding: data-parallel over batch. 16 samples / 8 cores = 2 samples/core.
All weights replicated (tiny).

Per-core pipeline (spatial 56x56 = 3136 flattened, S=2 samples):
  conv1 (1x1, 256->64) +BN1+ReLU   : PE matmuls (bf16), ACT evac with fused
      scale(folded)+bias+relu, written into zero-padded 62-wide row planes
      (out1p) so the involution halo gather needs no edge special-casing.
  inv_c1 (1x1, 64->16) +BN+ReLU    : PE, ACT evac -> z
  inv_c2 (1x1, 16->196) + bias     : PE (two 98-wide halves, group-aligned),
      ACT evac -> dynamic weights w2a/w2b
  involution (G=4, 7x7 dynamic)    : the 49 per-tap products run on DVE
      (tensor_tensor mult, bf16 2x mode; 41 taps) and GpSimd (8 taps) in a
      (sample, group, 4-row-chunk) partition layout (112 partitions); the
      accumulation adds run on the otherwise-idle Tensor engine as identity
      matmuls into a 7-bank fp32 PSUM region. BN2's bias (pre-divided by its
      scale) is added by a K=4 group-indicator matmul; its scale folds into
      conv3's weights, so the PSUM evacuation IS the BN2+ReLU.
  conv3 (1x1, 64->256) +BN3 + residual + ReLU : PE (residual folded in as an
      identity matmul over bf16 input), paired 896-wide chunks, ACT evac w/
      fused bias+relu.

Corner-turns bounce through DRAM staging tiles (SBUF APs can only carry the
partition dim as their leading dim). The xh path rides the sync HWDGE ring,
the w2 path the scalar ring; w2t gathers are split per (s, g, kh) and the tap
loop runs kh-major so the DVE starts after the first kh group instead of the
whole gather. PSUM pools open sequentially (convs / 7-bank accumulator /
conv3) so chunks pipeline instead of serializing on whole-tile WARs.

Compute dtype bf16 (f32 PSUM accumulation); output f32.
"""

import sys

sys.path.insert(0, "/opt/trn_rl_repo")

import numpy as np
import ml_dtypes

BF16 = ml_dtypes.bfloat16

S = 2            # samples per core
N_CORES = 8
CIN = 256
CMID = 64
G = 4            # involution groups
GC = 16          # channels per group
KS = 7           # involution kernel size
KK = KS * KS     # 49
R = 16           # dyn-weight bottleneck channels
H = W = 56
HW = H * W       # 3136
NCH = 7          # spatial chunks for matmul N dim (448 positions = 8 rows)
NW = HW // NCH   # 448
M = 14           # 4-row chunks per (sample, group)
RH = 4           # output rows per chunk
HR = 10          # halo rows stored per chunk (-3..+6)
WP = 62          # padded row width
PR = 65          # padded rows per plane (-3..61)
PLANE = PR * WP  # 4030 elems per (sample, channel) plane
NP_INV = S * G * M          # 112 involution partitions
XHF = GC * HR * WP          # 9920 free elems per XH partition
W2F = KK * RH * W           # 10976 free elems per W2 partition
ACCF = GC * RH * W           # 3584 acc free elems per partition
NBANK = 7                   # psum bank-chunks (512 f32 each)
EPS = 1e-5

# GpSimd tap offload measured as a loss: a DVE tensor_tensor's second operand
# rides the shared DVE/GpSimd SBUF port, so concurrent GpSimd tensor ops fully
# serialize against the DVE mul stream (plus a ~6us ucode IRAM load).
GP_TAPS = set()

_CACHE = {}


def _ap(tile_ap, off, dims):
    """Raw strided AP on a tile's underlying tensor. dims=[(step,count),...]
    in elements; for SBUF dims[0] must be the partition dim (step = the
    tile AP's leading stride)."""
    import bass_rust

    return bass_rust.AP(tile_ap.tensor, tile_ap.offset + off, [list(d) for d in dims])


def build_module():
    if "nc" in _CACHE:
        return _CACHE["nc"]
    import concourse.bacc as bacc
    import concourse.mybir as mybir
    import concourse.tile as tile

    dt = mybir.dt
    AF = mybir.ActivationFunctionType

    nc = bacc.Bacc("TRN2", debug=False, num_devices=N_CORES)

    # ---- DRAM I/O ----------------------------------------------------------
    xin = nc.dram_tensor("xin", [S, CIN, HW], dt.float32, kind="ExternalInput")
    w1t = nc.dram_tensor("w1t", [2, 128, CMID], dt.bfloat16, kind="ExternalInput")
    b1 = nc.dram_tensor("b1", [CMID, 1], dt.float32, kind="ExternalInput")
    c1t = nc.dram_tensor("c1t", [CMID, R], dt.bfloat16, kind="ExternalInput")
    bi = nc.dram_tensor("bi", [R, 1], dt.float32, kind="ExternalInput")
    # inv_c2 halves carry their bias as a 17th row (matched by a ones row in z)
    c2ta = nc.dram_tensor("c2ta", [R + 1, 98], dt.bfloat16, kind="ExternalInput")
    c2tb = nc.dram_tensor("c2tb", [R + 1, 98], dt.bfloat16, kind="ExternalInput")
    b2pat = nc.dram_tensor("b2pat", [G, ACCF], dt.bfloat16, kind="ExternalInput")
    zone = nc.dram_tensor("zone", [1, S * HW], dt.bfloat16, kind="ExternalInput")
    gsel = nc.dram_tensor("gsel", [G, 128], dt.bfloat16, kind="ExternalInput")
    w3t = nc.dram_tensor("w3t", [2, CMID, 128], dt.bfloat16, kind="ExternalInput")
    b3 = nc.dram_tensor("b3", [128, 2], dt.float32, kind="ExternalInput")
    ident = nc.dram_tensor("ident", [128, 128], dt.bfloat16, kind="ExternalInput")
    out = nc.dram_tensor("out", [S, CIN, HW], dt.float32, kind="ExternalOutput")

    with tile.TileContext(nc) as tc:
        with (
            tc.tile_pool(name="consts", bufs=1) as cpool,
            tc.tile_pool(name="big", bufs=1) as bpool,
            tc.tile_pool(name="tmp", bufs=4) as tpool,
            tc.tile_pool(name="stage", bufs=3) as spool,
            tc.tile_pool(name="dstage", bufs=1, space="DRAM") as dpool,
        ):
            # ---- constants -> SBUF ----------------------------------------
            w1t_sb = cpool.tile([128, 2 * CMID], dt.bfloat16, tag="w1t")
            nc.sync.dma_start(
                out=w1t_sb[:, :].rearrange("p (k c) -> p k c", k=2),
                in_=w1t.ap().rearrange("k p c -> p k c"),
            )
            b1_sb = cpool.tile([CMID, 1], dt.float32, tag="b1")
            nc.sync.dma_start(out=b1_sb[:, :], in_=b1.ap())
            c1t_sb = cpool.tile([CMID, R], dt.bfloat16, tag="c1t")
            nc.sync.dma_start(out=c1t_sb[:, :], in_=c1t.ap())
            bi_sb = cpool.tile([R, 1], dt.float32, tag="bi")
            nc.sync.dma_start(out=bi_sb[:, :], in_=bi.ap())
            c2ta_sb = cpool.tile([R + 1, 98], dt.bfloat16, tag="c2ta")
            nc.sync.dma_start(out=c2ta_sb[:, :], in_=c2ta.ap())
            c2tb_sb = cpool.tile([R + 1, 98], dt.bfloat16, tag="c2tb")
            nc.sync.dma_start(out=c2tb_sb[:, :], in_=c2tb.ap())
            b2pat_sb = cpool.tile([G, ACCF], dt.bfloat16, tag="b2pat")
            nc.sync.dma_start(out=b2pat_sb[:, :], in_=b2pat.ap())
            gsel_sb = cpool.tile([G, 128], dt.bfloat16, tag="gsel")
            nc.sync.dma_start(out=gsel_sb[:, :], in_=gsel.ap())
            w3t_sb = cpool.tile([CMID, 2 * 128], dt.bfloat16, tag="w3t")
            nc.sync.dma_start(
                out=w3t_sb[:, :].rearrange("p (k c) -> p k c", k=2),
                in_=w3t.ap().rearrange("k p c -> p k c"),
            )
            b3_sb = cpool.tile([128, 2], dt.float32, tag="b3")
            nc.sync.dma_start(out=b3_sb[:, :], in_=b3.ap())
            id_sb = cpool.tile([128, 128], dt.bfloat16, tag="ident")
            nc.sync.dma_start(out=id_sb[:, :], in_=ident.ap())

            # ---- big SBUF tensors -----------------------------------------
            xbf = bpool.tile([128, S * 2 * HW], dt.bfloat16, tag="xbf")
            xbf_v = xbf[:, :].rearrange("p (s k f) -> p s k f", s=S, k=2)
            out1p = bpool.tile([CMID, S * PLANE], dt.bfloat16, tag="out1p")
            o1p_v = out1p[:, :].rearrange("p (s r w) -> p s r w", s=S, w=WP)
            z_sb = bpool.tile([R + 1, S * HW], dt.bfloat16, tag="z")
            w2ab = bpool.tile([98, 2 * S * HW], dt.bfloat16, tag="w2ab")
            w2ab_ap = w2ab[:, :]
            P_W2AB = w2ab_ap.ap[0][0]
            xh = bpool.tile([NP_INV, XHF], dt.bfloat16, tag="xh")
            xh2 = bpool.tile([NP_INV, XHF], dt.bfloat16, tag="xh2")
            w2t = bpool.tile([NP_INV, W2F], dt.bfloat16, tag="w2t")
            accsb = bpool.tile([NP_INV, ACCF], dt.bfloat16, tag="accsb")
            out2 = bpool.tile([CMID, S * HW], dt.bfloat16, tag="out2")

            o1d = dpool.tile([S * CMID, PLANE], dt.bfloat16, tag="o1d")
            o1d_ap = o1d[:, :]
            w2d = dpool.tile([S * 2 * 98, HW], dt.bfloat16, tag="w2d")
            w2d_ap = w2d[:, :]
            o1p_ap = out1p[:, :]
            P_O1P = o1p_ap.ap[0][0]
            xh_ap = xh[:, :]
            P_XH = xh_ap.ap[0][0]
            w2t_ap = w2t[:, :]
            P_W2T = w2t_ap.ap[0][0]

            # ones row of z (meets the bias row of c2ta/c2tb); DMA-filled
            # because engine ops can't target a partition-16 base
            nc.sync.dma_start(out=z_sb[R : R + 1, :], in_=zone.ap())
            # zero only out1p's pad regions: top halo rows, bottom halo rows,
            # and the 6-wide inter-row pad runs of the data rows
            for s in range(S):
                nc.vector.memset(
                    _ap(o1p_ap, s * PLANE, [(P_O1P, CMID), (1, 3 * WP + 3)]), 0.0
                )
                nc.vector.memset(
                    _ap(o1p_ap, s * PLANE + 59 * WP, [(P_O1P, CMID), (1, 6 * WP)]),
                    0.0,
                )
                nc.vector.memset(
                    _ap(
                        o1p_ap,
                        s * PLANE + 3 * WP + 3 + W,
                        [(P_O1P, CMID), (WP, H), (1, 6)],
                    ),
                    0.0,
                )

            # ---- x load (f32 -> bf16 cast during SWDGE DMA), (s, kc) ------
            for s in range(S):
                for kc in range(2):
                    nc.gpsimd.dma_start(
                        out=xbf_v[:, s, kc],
                        in_=xin.ap()[s, kc * 128 : (kc + 1) * 128],
                    )

            w1t_v = w1t_sb[:, :].rearrange("p (k c) -> p k c", k=2)

            # ---- conv chain, per sample; staging DMAs interleaved ---------
            with (
                tc.tile_pool(name="psc", bufs=4, space="PSUM") as pcv,
                tc.tile_pool(name="psc2", bufs=2, space="PSUM") as pc2,
            ):
                for s in range(S):
                    # conv1 + BN1 + ReLU -> out1p (padded planes)
                    for n in range(NCH):
                        ps = pcv.tile([CMID, NW], dt.float32, tag="ps1")
                        for kc in range(2):
                            nc.tensor.matmul(
                                ps[:, :],
                                w1t_v[:, kc, :],
                                xbf_v[:, s, kc, n * NW : (n + 1) * NW],
                                start=(kc == 0),
                                stop=(kc == 1),
                            )
                        nc.scalar.activation(
                            o1p_v[:, s, 3 + 8 * n : 3 + 8 * n + 8, 3 : 3 + W],
                            ps[:, :].rearrange("p (r w) -> p r w", r=8),
                            AF.Relu,
                            bias=b1_sb[:, 0:1],
                        )
                    # out1p(s) -> o1d(s), then xh(s, g) gathers (sync ring)
                    nc.sync.dma_start(
                        out=_ap(o1d_ap, s * CMID * PLANE, [(PLANE, CMID), (1, PLANE)]),
                        in_=_ap(o1p_ap, s * PLANE, [(P_O1P, CMID), (1, PLANE)]),
                    )
                    for g in range(G):
                        pb = (s * G + g) * M
                        nc.sync.dma_start(
                            out=_ap(
                                xh_ap,
                                pb * P_XH,
                                [(P_XH, M), (HR * WP, GC), (1, HR * WP)],
                            ),
                            in_=_ap(
                                o1d_ap,
                                s * CMID * PLANE + (g * GC) * PLANE,
                                [(RH * WP, M), (PLANE, GC), (1, HR * WP)],
                            ),
                        )
                    # inv_c1 + BN + ReLU -> z
                    for n in range(NCH):
                        ps = pcv.tile([R, NW], dt.float32, tag="ps1")
                        nc.tensor.matmul(
                            ps[:, :],
                            c1t_sb[:, :],
                            o1p_v[:, s, 3 + 8 * n : 3 + 8 * n + 8, 3 : 3 + W],
                            start=True,
                            stop=True,
                        )
                        nc.scalar.activation(
                            z_sb[:R, s * HW + n * NW : s * HW + (n + 1) * NW],
                            ps[:, :],
                            AF.Relu,
                            bias=bi_sb[:, 0:1],
                        )
                    # inv_c2 (+bias via the ones row) -> w2ab halves, one
                    # paired evac per chunk
                    for n in range(NCH):
                        zsl = z_sb[:, s * HW + n * NW : s * HW + (n + 1) * NW]
                        psab = pc2.tile([98, 1024], dt.float32, tag="ps2")
                        nc.tensor.matmul(
                            psab[:, 0:NW], c2ta_sb[:, :], zsl, start=True, stop=True
                        )
                        nc.tensor.matmul(
                            psab[:, 512 : 512 + NW],
                            c2tb_sb[:, :],
                            zsl,
                            start=True,
                            stop=True,
                        )
                        psab_ap = psab[:, :]
                        nc.scalar.activation(
                            _ap(
                                w2ab_ap,
                                s * HW + n * NW,
                                [(P_W2AB, 98), (S * HW, 2), (1, NW)],
                            ),
                            _ap(psab_ap, 0, [(psab_ap.ap[0][0], 98), (512, 2), (1, NW)]),
                            AF.Identity,
                        )
                    # w2(s) -> w2d(s) [ko, hw] (scalar ring; emitted after the
                    # evacs above, so the ACT FIFO isn't blocked by its waits)
                    for half in (0, 1):
                        nc.scalar.dma_start(
                            out=_ap(
                                w2d_ap,
                                s * 2 * 98 * HW + half * 98 * HW,
                                [(HW, 98), (1, HW)],
                            ),
                            in_=_ap(
                                w2ab_ap,
                                half * S * HW + s * HW,
                                [(P_W2AB, 98), (1, HW)],
                            ),
                        )

                # xh2 = xh shifted right one element (sync ring; xh2[:, 0] and
                # the first two columns of every row are never read)
                xh2_ap = xh2[:, :]
                P_XH2 = xh2_ap.ap[0][0]
                nc.sync.dma_start(
                    out=_ap(xh2_ap, 1, [(P_XH2, NP_INV), (1, XHF - 1)]),
                    in_=_ap(xh_ap, 0, [(P_XH, NP_INV), (1, XHF - 1)]),
                )
                # w2t gathers, kh-major so the tap loop can chase them;
                # alternate HWDGE rings to double gather throughput
                ring = [nc.sync, nc.scalar]
                for kh in range(KS):
                    for s in range(S):
                        for g in range(G):
                            pb = (s * G + g) * M
                            ring[(kh * S * G + s * G + g) % 2].dma_start(
                                out=_ap(
                                    w2t_ap,
                                    pb * P_W2T + kh * KS * RH * W,
                                    [(P_W2T, M), (RH * W, KS), (1, RH * W)],
                                ),
                                in_=_ap(
                                    w2d_ap,
                                    s * 2 * 98 * HW + g * KK * HW + kh * KS * HW,
                                    [(RH * W, M), (HW, KS), (1, RH * W)],
                                ),
                            )

            # ---- involution: DVE/GpSimd products + PE matmul accumulate ---
            xh_v = xh[:, :].rearrange("p (c r w) -> p c r w", r=HR, w=WP)
            xh2_v = xh2[:, :].rearrange("p (c r w) -> p c r w", r=HR, w=WP)
            w2t_v = w2t[:, :].rearrange("p (k r w) -> p k r w", k=KK, r=RH)
            # kh-major (chasing the w2t gathers); evens before odds within a
            # row (odd-kw taps read xh2, which lands after xh)
            taps = []
            for kh in range(KS):
                row = list(range(kh * KS, (kh + 1) * KS))
                taps += [k for k in row if (k % KS) % 2 == 0]
                taps += [k for k in row if (k % KS) % 2 == 1]
            with tc.tile_pool(name="psa", bufs=1, space="PSUM") as pac:
                acc = pac.tile([128, NBANK * 512], dt.float32, tag="acc")
                for i, k in enumerate(taps):
                    kh, kw = divmod(k, KS)
                    if kw % 2 == 0:
                        src_v, wc = xh_v, kw
                    else:
                        src_v, wc = xh2_v, kw + 1
                    tmp = tpool.tile([NP_INV, ACCF], dt.bfloat16, tag="tmp")
                    tmp_v = tmp[:, :].rearrange("p (c r w) -> p c r w", r=RH, w=W)
                    last_tmp = tmp
                    eng = nc.gpsimd if k in GP_TAPS else nc.vector
                    eng.tensor_mul(
                        tmp_v,
                        src_v[:, :, kh : kh + RH, wc : wc + W],
                        w2t_v[:, k : k + 1, :, :].to_broadcast([NP_INV, GC, RH, W]),
                    )
                    for n in range(NBANK):
                        nc.tensor.matmul(
                            acc[:NP_INV, n * 512 : (n + 1) * 512],
                            id_sb[:NP_INV, :NP_INV],
                            tmp[:, n * 512 : (n + 1) * 512],
                            start=(i == 0),
                            stop=False,
                            skip_group_check=True,
                        )
                # BN2 bias (pre-divided by scale) via a K=4 matmul: the
                # group-indicator stationary routes pattern g to partitions
                # of group g
                for n in range(NBANK):
                    nc.tensor.matmul(
                        acc[:NP_INV, n * 512 : (n + 1) * 512],
                        gsel_sb[:, :NP_INV],
                        b2pat_sb[:, n * 512 : (n + 1) * 512],
                        start=False,
                        stop=True,
                        skip_group_check=True,
                    )
                # evac IS the BN2 ReLU (scale folded into conv3 weights)
                nc.scalar.activation(accsb[:, :], acc[:NP_INV, :ACCF], AF.Relu)

                # ---- corner-turn back to channel partitions, both rings ---
                accd = dpool.tile([NP_INV, ACCF], dt.bfloat16, tag="accd")
                accd_ap = accd[:, :]
                accsb_ap = accsb[:, :]
                P_ASB = accsb_ap.ap[0][0]
                out2_ap = out2[:, :]
                P_O2 = out2_ap.ap[0][0]
                for s in range(S):
                    eng = nc.sync if s == 0 else nc.scalar
                    eng.dma_start(
                        out=_ap(
                            accd_ap, s * (G * M) * ACCF, [(ACCF, G * M), (1, ACCF)]
                        ),
                        in_=_ap(
                            accsb_ap, s * (G * M) * P_ASB, [(P_ASB, G * M), (1, ACCF)]
                        ),
                    )
                    for g in range(G):
                        pb = (s * G + g) * M
                        eng.dma_start(
                            out=_ap(
                                out2_ap,
                                (g * GC) * P_O2 + s * HW,
                                [(P_O2, GC), (RH * W, M), (1, RH * W)],
                            ),
                            in_=_ap(
                                accd_ap,
                                pb * ACCF,
                                [(RH * W, GC), (ACCF, M), (1, RH * W)],
                            ),
                        )

                # keep the PE HAM-warm across the evac/corner-turn window so
                # conv3 doesn't run at the cold 1.2 GHz clock: a burst of junk
                # matmuls into a scratch bank, then two gated on the evac and
                # the sample-0 gathers
                scratch = pac.tile([128, 512], dt.float32, tag="warm")
                for j in range(16):
                    nc.tensor.matmul(
                        scratch[:NP_INV, :],
                        id_sb[:NP_INV, :NP_INV],
                        last_tmp[:, 0:512],
                        start=True,
                        stop=True,
                        skip_group_check=True,
                    )
                nc.tensor.matmul(
                    scratch[:NP_INV, :],
                    id_sb[:NP_INV, :NP_INV],
                    accsb[:, 0:512],
                    start=True,
                    stop=True,
                    skip_group_check=True,
                )
                nc.tensor.matmul(
                    scratch[:CMID, :],
                    id_sb[:CMID, :CMID],
                    out2[:, 0:512],
                    start=True,
                    stop=True,
                    skip_group_check=True,
                )

            # ---- conv3 + BN3 + residual + ReLU -> out (paired chunks) -----
            w3t_v = w3t_sb[:, :].rearrange("p (k c) -> p k c", k=2)
            with tc.tile_pool(name="ps3", bufs=3, space="PSUM") as p3:
                for s in range(S):
                    for oc in range(2):
                        for n0 in (0, 2, 4, 6):
                            npair = 1 if n0 == 6 else 2
                            ps = p3.tile([128, 1024], dt.float32, tag="ps3")
                            for j in range(npair):
                                n = n0 + j
                                psl = ps[:, j * 512 : j * 512 + NW]
                                nc.tensor.matmul(
                                    psl,
                                    w3t_v[:, oc, :],
                                    out2[:, s * HW + n * NW : s * HW + (n + 1) * NW],
                                    start=True,
                                    stop=False,
                                )
                                nc.tensor.matmul(
                                    psl,
                                    id_sb[:, :],
                                    xbf_v[:, s, oc, n * NW : (n + 1) * NW],
                                    start=False,
                                    stop=True,
                                )
                            ob = spool.tile([128, 2 * NW], dt.float32, tag="obuf")
                            nc.scalar.activation(
                                ob[:, : npair * NW],
                                _ap(
                                    ps[:, :],
                                    0,
                                    [(ps[:, :].ap[0][0], 128), (512, npair), (1, NW)],
                                ),
                                AF.Relu,
                                bias=b3_sb[:, oc : oc + 1],
                            )
                            nc.sync.dma_start(
                                out=out.ap()[
                                    s,
                                    oc * 128 : (oc + 1) * 128,
                                    n0 * NW : (n0 + npair) * NW,
                                ],
                                in_=ob[:, : npair * NW],
                            )

    nc.compile()
    _CACHE["nc"] = nc
    return nc


def _f32(a):
    return np.ascontiguousarray(a, dtype=np.float32)


def prep_weights(inputs):
    """Host-side folding of BN scales into conv weights; bf16 casts."""
    f = inputs
    s1 = f["bn1_g"] / np.sqrt(f["bn1_v"] + EPS)
    b1_eff = f["bn1_b"] - f["bn1_m"] * s1
    w1t_eff = (_f32(f["conv1_w"]) * s1[:, None]).T          # [256, 64]

    si = f["inv_bn_g"] / np.sqrt(f["inv_bn_v"] + EPS)
    bi_eff = f["inv_bn_b"] - f["inv_bn_m"] * si
    c1t_eff = (_f32(f["inv_c1_w"]) * si[:, None]).T         # [64, 16]

    c2t_eff = _f32(f["inv_c2_w"]).T                         # [16, 196]
    b2c = _f32(f["inv_c2_b"])

    # relu(s2*y + b2n) = s2 * relu(y + b2n/s2), valid because s2 > 0: the
    # scale folds into conv3's input columns, the shifted bias is added in
    # PSUM by the gsel/b2pat matmul, and the accumulator evac applies the relu
    s2 = _f32(f["bn2_g"] / np.sqrt(f["bn2_v"] + EPS))
    b2n = _f32(f["bn2_b"] - f["bn2_m"] * s2)
    s3 = f["bn3_g"] / np.sqrt(f["bn3_v"] + EPS)
    b3_eff = f["bn3_b"] - f["bn3_m"] * s3
    w3_eff = _f32(f["conv3_w"]) * s3[:, None] * s2[None, :]  # [256, 64]
    w3t_eff = w3_eff.T                                       # [64, 256]

    d = {}
    d["w1t"] = np.ascontiguousarray(
        w1t_eff.reshape(2, 128, CMID).astype(BF16)
    )
    d["b1"] = _f32(b1_eff)[:, None]
    d["c1t"] = np.ascontiguousarray(c1t_eff.astype(BF16))
    d["bi"] = _f32(bi_eff)[:, None]
    c2ta_x = np.concatenate([c2t_eff[:, 0:98], b2c[None, 0:98]], axis=0)
    c2tb_x = np.concatenate([c2t_eff[:, 98:196], b2c[None, 98:196]], axis=0)
    d["c2ta"] = np.ascontiguousarray(c2ta_x.astype(BF16))
    d["c2tb"] = np.ascontiguousarray(c2tb_x.astype(BF16))
    d["w3t"] = np.ascontiguousarray(
        w3t_eff.reshape(CMID, 2, 128).transpose(1, 0, 2).astype(BF16)
    )
    d["b3"] = _f32(b3_eff.reshape(2, 128).T)
    d["ident"] = np.ascontiguousarray(np.eye(128, dtype=np.float32).astype(BF16))
    b2r = b2n / s2           # relu-shifted BN2 bias per mid channel [64]
    d["b2pat"] = np.ascontiguousarray(
        np.repeat(b2r.reshape(G, GC), RH * W, axis=1).astype(BF16)
    )
    d["zone"] = np.ones((1, S * HW), dtype=BF16)
    gs = np.zeros((G, 128), np.float32)
    for p in range(NP_INV):
        gs[(p // M) % G, p] = 1.0
    d["gsel"] = np.ascontiguousarray(gs.astype(BF16))
    return d


def make_in_maps(inputs):
    prep = prep_weights(inputs)
    x = _f32(inputs["x"]).reshape(16, CIN, HW)
    in_maps = []
    for i in range(N_CORES):
        m = dict(prep)
        m["xin"] = np.ascontiguousarray(x[S * i : S * i + S])
        in_maps.append(m)
    return in_maps


def kernel(**inputs):
    from concourse.bass_utils import run_bass_kernel_spmd

    nc = build_module()
    in_maps = make_in_maps(inputs)
    res = run_bass_kernel_spmd(nc, in_maps, core_ids=list(range(N_CORES)))
    outs = [res.results[i]["out"].reshape(S, CIN, H, W) for i in range(N_CORES)]
    return np.concatenate(outs, axis=0).astype(np.float32)


# revision 31
# speedup vs baseline: 1.1855x; 1.1855x over previous
"""Trainium2 Bass kernel for nn_Bottleneck_5669356834470 (ResNet bottleneck
with an involution middle layer).

Sharding: data-parallel over batch. 16 samples / 8 cores = 2 samples/core.
All weights replicated (tiny).

Per-core pipeline (spatial 56x56 = 3136 flattened, S=2 samples):
  conv1 (1x1, 256->64) +BN1+ReLU   : PE matmuls (bf16), ACT evac with fused
      scale(folded)+bias+relu, written into zero-padded 62-wide row planes
      (out1p) so the involution halo gather needs no edge special-casing.
  inv_c1 (1x1, 64->16) +BN+ReLU    : PE, ACT evac -> z
  inv_c2 (1x1, 16->196) + bias     : PE (two 98-wide halves, group-aligned),
      ACT evac -> dynamic weights w2a/w2b
  involution (G=4, 7x7 dynamic)    : the 49 per-tap products run on DVE
      (tensor_tensor mult, bf16 2x mode; 41 taps) and GpSimd (8 taps) in a
      (sample, group, 4-row-chunk) partition layout (112 partitions); the
      accumulation adds run on the otherwise-idle Tensor engine as identity
      matmuls into a 7-bank fp32 PSUM region. BN2's bias (pre-divided by its
      scale) is added by a K=4 group-indicator matmul; its scale folds into
      conv3's weights, so the PSUM evacuation IS the BN2+ReLU.
  conv3 (1x1, 64->256) +BN3 + residual + ReLU : PE (residual folded in as an
      identity matmul over bf16 input), paired 896-wide chunks, ACT evac w/
      fused bias+relu.

Corner-turns bounce through DRAM staging tiles (SBUF APs can only carry the
partition dim as their leading dim). The xh path rides the sync HWDGE ring,
the w2 path the scalar ring; w2t gathers are split per (s, g, kh) and the tap
loop runs kh-major so the DVE starts after the first kh group instead of the
whole gather. PSUM pools open sequentially (convs / 7-bank accumulator /
conv3) so chunks pipeline instead of serializing on whole-tile WARs.

Compute dtype bf16 (f32 PSUM accumulation); output f32.
"""

import sys

sys.path.insert(0, "/opt/trn_rl_repo")

import numpy as np
import ml_dtypes

BF16 = ml_dtypes.bfloat16

S = 2            # samples per core
N_CORES = 8
CIN = 256
CMID = 64
G = 4            # involution groups
GC = 16          # channels per group
KS = 7           # involution kernel size
KK = KS * KS     # 49
R = 16           # dyn-weight bottleneck channels
H = W = 56
HW = H * W       # 3136
NCH = 7          # spatial chunks for matmul N dim (448 positions = 8 rows)
NW = HW // NCH   # 448
M = 14           # 4-row chunks per (sample, group)
RH = 4           # output rows per chunk
HR = 10          # halo rows stored per chunk (-3..+6)
WP = 62          # padded row width
PR = 65          # padded rows per plane (-3..61)
PLANE = PR * WP  # 4030 elems per (sample, channel) plane
NP_INV = S * G * M          # 112 involution partitions
XHF = GC * HR * WP          # 9920 free elems per XH partition
W2F = KK * RH * W           # 10976 free elems per W2 partition
ACCF = GC * RH * W           # 3584 acc free elems per partition
NBANK = 7                   # psum bank-chunks (512 f32 each)
EPS = 1e-5

# GpSimd tap offload measured as a loss: a DVE tensor_tensor's second operand
# rides the shared DVE/GpSimd SBUF port, so concurrent GpSimd tensor ops fully
# serialize against the DVE mul stream (plus a ~6us ucode IRAM load).
GP_TAPS = set()

_CACHE = {}


def _ap(tile_ap, off, dims):
    """Raw strided AP on a tile's underlying tensor. dims=[(step,count),...]
    in elements; for SBUF dims[0] must be the partition dim (step = the
    tile AP's leading stride)."""
    import bass_rust

    return bass_rust.AP(tile_ap.tensor, tile_ap.offset + off, [list(d) for d in dims])


def build_module():
    if "nc" in _CACHE:
        return _CACHE["nc"]
    import concourse.bacc as bacc
    import concourse.mybir as mybir
    import concourse.tile as tile

    dt = mybir.dt
    AF = mybir.ActivationFunctionType

    nc = bacc.Bacc("TRN2", debug=False, num_devices=N_CORES)

    # ---- DRAM I/O ----------------------------------------------------------
    xin = nc.dram_tensor("xin", [S, CIN, HW], dt.float32, kind="ExternalInput")
    w1t = nc.dram_tensor("w1t", [2, 128, CMID], dt.bfloat16, kind="ExternalInput")
    b1 = nc.dram_tensor("b1", [CMID, 1], dt.float32, kind="ExternalInput")
    c1t = nc.dram_tensor("c1t", [CMID, R], dt.bfloat16, kind="ExternalInput")
    bi = nc.dram_tensor("bi", [R, 1], dt.float32, kind="ExternalInput")
    # inv_c2 halves carry their bias as a 17th row (matched by a ones row in z)
    c2ta = nc.dram_tensor("c2ta", [R + 1, 98], dt.bfloat16, kind="ExternalInput")
    c2tb = nc.dram_tensor("c2tb", [R + 1, 98], dt.bfloat16, kind="ExternalInput")
    b2pat = nc.dram_tensor("b2pat", [G, ACCF], dt.bfloat16, kind="ExternalInput")
    zone = nc.dram_tensor("zone", [1, S * HW], dt.bfloat16, kind="ExternalInput")
    gsel = nc.dram_tensor("gsel", [G, 128], dt.bfloat16, kind="ExternalInput")
    w3t = nc.dram_tensor("w3t", [2, CMID, 128], dt.bfloat16, kind="ExternalInput")
    b3 = nc.dram_tensor("b3", [128, 2], dt.float32, kind="ExternalInput")
    ident = nc.dram_tensor("ident", [128, 128], dt.bfloat16, kind="ExternalInput")
    out = nc.dram_tensor("out", [S, CIN, HW], dt.float32, kind="ExternalOutput")

    with tile.TileContext(nc) as tc:
        with (
            tc.tile_pool(name="consts", bufs=1) as cpool,
            tc.tile_pool(name="big", bufs=1) as bpool,
            tc.tile_pool(name="tmp", bufs=4) as tpool,
            tc.tile_pool(name="stage", bufs=4) as spool,
            tc.tile_pool(name="dstage", bufs=1, space="DRAM") as dpool,
        ):
            # ---- constants -> SBUF ----------------------------------------
            w1t_sb = cpool.tile([128, 2 * CMID], dt.bfloat16, tag="w1t")
            nc.sync.dma_start(
                out=w1t_sb[:, :].rearrange("p (k c) -> p k c", k=2),
                in_=w1t.ap().rearrange("k p c -> p k c"),
            )
            b1_sb = cpool.tile([CMID, 1], dt.float32, tag="b1")
            nc.sync.dma_start(out=b1_sb[:, :], in_=b1.ap())
            c1t_sb = cpool.tile([CMID, R], dt.bfloat16, tag="c1t")
            nc.sync.dma_start(out=c1t_sb[:, :], in_=c1t.ap())
            bi_sb = cpool.tile([R, 1], dt.float32, tag="bi")
            nc.sync.dma_start(out=bi_sb[:, :], in_=bi.ap())
            c2ta_sb = cpool.tile([R + 1, 98], dt.bfloat16, tag="c2ta")
            nc.sync.dma_start(out=c2ta_sb[:, :], in_=c2ta.ap())
            c2tb_sb = cpool.tile([R + 1, 98], dt.bfloat16, tag="c2tb")
            nc.sync.dma_start(out=c2tb_sb[:, :], in_=c2tb.ap())
            b2pat_sb = cpool.tile([G, ACCF], dt.bfloat16, tag="b2pat")
            nc.sync.dma_start(out=b2pat_sb[:, :], in_=b2pat.ap())
            gsel_sb = cpool.tile([G, 128], dt.bfloat16, tag="gsel")
            nc.sync.dma_start(out=gsel_sb[:, :], in_=gsel.ap())
            w3t_sb = cpool.tile([CMID, 2 * 128], dt.bfloat16, tag="w3t")
            nc.sync.dma_start(
                out=w3t_sb[:, :].rearrange("p (k c) -> p k c", k=2),
                in_=w3t.ap().rearrange("k p c -> p k c"),
            )
            b3_sb = cpool.tile([128, 2], dt.float32, tag="b3")
            nc.sync.dma_start(out=b3_sb[:, :], in_=b3.ap())
            id_sb = cpool.tile([128, 128], dt.bfloat16, tag="ident")
            nc.sync.dma_start(out=id_sb[:, :], in_=ident.ap())

            # ---- big SBUF tensors -----------------------------------------
            xbf = bpool.tile([128, S * 2 * HW], dt.bfloat16, tag="xbf")
            xbf_v = xbf[:, :].rearrange("p (s k f) -> p s k f", s=S, k=2)
            out1p = bpool.tile([CMID, S * PLANE], dt.bfloat16, tag="out1p")
            o1p_v = out1p[:, :].rearrange("p (s r w) -> p s r w", s=S, w=WP)
            z_sb = bpool.tile([R + 1, S * HW], dt.bfloat16, tag="z")
            w2ab = bpool.tile([98, 2 * S * HW], dt.bfloat16, tag="w2ab")
            w2ab_ap = w2ab[:, :]
            P_W2AB = w2ab_ap.ap[0][0]
            xh = bpool.tile([NP_INV, XHF], dt.bfloat16, tag="xh")
            xh2 = bpool.tile([NP_INV, XHF], dt.bfloat16, tag="xh2")
            w2t = bpool.tile([NP_INV, W2F], dt.bfloat16, tag="w2t")
            accsb = bpool.tile([NP_INV, ACCF], dt.bfloat16, tag="accsb")
            out2 = bpool.tile([CMID, S * HW], dt.bfloat16, tag="out2")

            o1d = dpool.tile([S * CMID, PLANE], dt.bfloat16, tag="o1d")
            o1d_ap = o1d[:, :]
            w2d = dpool.tile([S * 2 * 98, HW], dt.bfloat16, tag="w2d")
            w2d_ap = w2d[:, :]
            o1p_ap = out1p[:, :]
            P_O1P = o1p_ap.ap[0][0]
            xh_ap = xh[:, :]
            P_XH = xh_ap.ap[0][0]
            w2t_ap = w2t[:, :]
            P_W2T = w2t_ap.ap[0][0]

            # ones row of z (meets the bias row of c2ta/c2tb); DMA-filled
            # because engine ops can't target a partition-16 base
            nc.sync.dma_start(out=z_sb[R : R + 1, :], in_=zone.ap())
            # zero only out1p's pad regions: top halo rows, bottom halo rows,
            # and the 6-wide inter-row pad runs of the data rows
            for s in range(S):
                nc.vector.memset(
                    _ap(o1p_ap, s * PLANE, [(P_O1P, CMID), (1, 3 * WP + 3)]), 0.0
                )
                nc.vector.memset(
                    _ap(o1p_ap, s * PLANE + 59 * WP, [(P_O1P, CMID), (1, 6 * WP)]),
                    0.0,
                )
                nc.vector.memset(
                    _ap(
                        o1p_ap,
                        s * PLANE + 3 * WP + 3 + W,
                        [(P_O1P, CMID), (WP, H), (1, 6)],
                    ),
                    0.0,
                )

            # ---- x load (f32 -> bf16 cast during SWDGE DMA), (s, kc) ------
            for s in range(S):
                for kc in range(2):
                    nc.gpsimd.dma_start(
                        out=xbf_v[:, s, kc],
                        in_=xin.ap()[s, kc * 128 : (kc + 1) * 128],
                    )

            w1t_v = w1t_sb[:, :].rearrange("p (k c) -> p k c", k=2)

            # ---- conv chain, per sample; staging DMAs interleaved ---------
            with (
                tc.tile_pool(name="psc", bufs=4, space="PSUM") as pcv,
                tc.tile_pool(name="psc2", bufs=2, space="PSUM") as pc2,
            ):
                for s in range(S):
                    # conv1 + BN1 + ReLU -> out1p (padded planes)
                    for n in range(NCH):
                        ps = pcv.tile([CMID, NW], dt.float32, tag="ps1")
                        for kc in range(2):
                            nc.tensor.matmul(
                                ps[:, :],
                                w1t_v[:, kc, :],
                                xbf_v[:, s, kc, n * NW : (n + 1) * NW],
                                start=(kc == 0),
                                stop=(kc == 1),
                            )
                        nc.scalar.activation(
                            o1p_v[:, s, 3 + 8 * n : 3 + 8 * n + 8, 3 : 3 + W],
                            ps[:, :].rearrange("p (r w) -> p r w", r=8),
                            AF.Relu,
                            bias=b1_sb[:, 0:1],
                        )
                    # out1p(s) -> o1d(s), then xh(s, g) gathers (sync ring)
                    nc.sync.dma_start(
                        out=_ap(o1d_ap, s * CMID * PLANE, [(PLANE, CMID), (1, PLANE)]),
                        in_=_ap(o1p_ap, s * PLANE, [(P_O1P, CMID), (1, PLANE)]),
                    )
                    for g in range(G):
                        pb = (s * G + g) * M
                        nc.sync.dma_start(
                            out=_ap(
                                xh_ap,
                                pb * P_XH,
                                [(P_XH, M), (HR * WP, GC), (1, HR * WP)],
                            ),
                            in_=_ap(
                                o1d_ap,
                                s * CMID * PLANE + (g * GC) * PLANE,
                                [(RH * WP, M), (PLANE, GC), (1, HR * WP)],
                            ),
                        )
                    # inv_c1 + BN + ReLU -> z
                    for n in range(NCH):
                        ps = pcv.tile([R, NW], dt.float32, tag="ps1")
                        nc.tensor.matmul(
                            ps[:, :],
                            c1t_sb[:, :],
                            o1p_v[:, s, 3 + 8 * n : 3 + 8 * n + 8, 3 : 3 + W],
                            start=True,
                            stop=True,
                        )
                        nc.scalar.activation(
                            z_sb[:R, s * HW + n * NW : s * HW + (n + 1) * NW],
                            ps[:, :],
                            AF.Relu,
                            bias=bi_sb[:, 0:1],
                        )
                    # inv_c2 (+bias via the ones row) -> w2ab halves, one
                    # paired evac per chunk
                    for n in range(NCH):
                        zsl = z_sb[:, s * HW + n * NW : s * HW + (n + 1) * NW]
                        psab = pc2.tile([98, 1024], dt.float32, tag="ps2")
                        nc.tensor.matmul(
                            psab[:, 0:NW], c2ta_sb[:, :], zsl, start=True, stop=True
                        )
                        nc.tensor.matmul(
                            psab[:, 512 : 512 + NW],
                            c2tb_sb[:, :],
                            zsl,
                            start=True,
                            stop=True,
                        )
                        psab_ap = psab[:, :]
                        nc.scalar.activation(
                            _ap(
                                w2ab_ap,
                                s * HW + n * NW,
                                [(P_W2AB, 98), (S * HW, 2), (1, NW)],
                            ),
                            _ap(psab_ap, 0, [(psab_ap.ap[0][0], 98), (512, 2), (1, NW)]),
                            AF.Identity,
                        )
                    # w2(s) -> w2d(s) [ko, hw] (scalar ring; emitted after the
                    # evacs above, so the ACT FIFO isn't blocked by its waits)
                    for half in (0, 1):
                        nc.scalar.dma_start(
                            out=_ap(
                                w2d_ap,
                                s * 2 * 98 * HW + half * 98 * HW,
                                [(HW, 98), (1, HW)],
                            ),
                            in_=_ap(
                                w2ab_ap,
                                half * S * HW + s * HW,
                                [(P_W2AB, 98), (1, HW)],
                            ),
                        )

                # xh2 = xh shifted right one element (sync ring; xh2[:, 0] and
                # the first two columns of every row are never read)
                xh2_ap = xh2[:, :]
                P_XH2 = xh2_ap.ap[0][0]
                nc.sync.dma_start(
                    out=_ap(xh2_ap, 1, [(P_XH2, NP_INV), (1, XHF - 1)]),
                    in_=_ap(xh_ap, 0, [(P_XH, NP_INV), (1, XHF - 1)]),
                )
                # w2t gathers, kh-major so the tap loop can chase them; all on
                # the scalar ring (the sync ring carries the xh chain — a
                # split queues half of each kh group behind xh2)
                for kh in range(KS):
                    for s in range(S):
                        for g in range(G):
                            pb = (s * G + g) * M
                            nc.scalar.dma_start(
                                out=_ap(
                                    w2t_ap,
                                    pb * P_W2T + kh * KS * RH * W,
                                    [(P_W2T, M), (RH * W, KS), (1, RH * W)],
                                ),
                                in_=_ap(
                                    w2d_ap,
                                    s * 2 * 98 * HW + g * KK * HW + kh * KS * HW,
                                    [(RH * W, M), (HW, KS), (1, RH * W)],
                                ),
                            )

            # ---- involution: DVE/GpSimd products + PE matmul accumulate ---
            xh_v = xh[:, :].rearrange("p (c r w) -> p c r w", r=HR, w=WP)
            xh2_v = xh2[:, :].rearrange("p (c r w) -> p c r w", r=HR, w=WP)
            w2t_v = w2t[:, :].rearrange("p (k r w) -> p k r w", k=KK, r=RH)
            # kh-major (chasing the w2t gathers); evens before odds within a
            # row (odd-kw taps read xh2, which lands after xh)
            taps = []
            for kh in range(KS):
                row = list(range(kh * KS, (kh + 1) * KS))
                taps += [k for k in row if (k % KS) % 2 == 0]
                taps += [k for k in row if (k % KS) % 2 == 1]
            with tc.tile_pool(name="psa", bufs=1, space="PSUM") as pac:
                acc = pac.tile([128, NBANK * 512], dt.float32, tag="acc")
                for i, k in enumerate(taps):
                    kh, kw = divmod(k, KS)
                    if kw % 2 == 0:
                        src_v, wc = xh_v, kw
                    else:
                        src_v, wc = xh2_v, kw + 1
                    tmp = tpool.tile([NP_INV, ACCF], dt.bfloat16, tag="tmp")
                    tmp_v = tmp[:, :].rearrange("p (c r w) -> p c r w", r=RH, w=W)
                    last_tmp = tmp
                    eng = nc.gpsimd if k in GP_TAPS else nc.vector
                    eng.tensor_mul(
                        tmp_v,
                        src_v[:, :, kh : kh + RH, wc : wc + W],
                        w2t_v[:, k : k + 1, :, :].to_broadcast([NP_INV, GC, RH, W]),
                    )
                    for n in range(NBANK):
                        nc.tensor.matmul(
                            acc[:NP_INV, n * 512 : (n + 1) * 512],
                            id_sb[:NP_INV, :NP_INV],
                            tmp[:, n * 512 : (n + 1) * 512],
                            start=(i == 0),
                            stop=False,
                            skip_group_check=True,
                        )
                # BN2 bias (pre-divided by scale) via a K=4 matmul: the
                # group-indicator stationary routes pattern g to partitions
                # of group g
                for n in range(NBANK):
                    nc.tensor.matmul(
                        acc[:NP_INV, n * 512 : (n + 1) * 512],
                        gsel_sb[:, :NP_INV],
                        b2pat_sb[:, n * 512 : (n + 1) * 512],
                        start=False,
                        stop=True,
                        skip_group_check=True,
                    )
                # evac IS the BN2 ReLU (scale folded into conv3 weights)
                nc.scalar.activation(accsb[:, :], acc[:NP_INV, :ACCF], AF.Relu)

                # ---- corner-turn back to channel partitions, both rings ---
                accd = dpool.tile([NP_INV, ACCF], dt.bfloat16, tag="accd")
                accd_ap = accd[:, :]
                accsb_ap = accsb[:, :]
                P_ASB = accsb_ap.ap[0][0]
                out2_ap = out2[:, :]
                P_O2 = out2_ap.ap[0][0]
                for s in range(S):
                    eng = nc.sync if s == 0 else nc.scalar
                    eng.dma_start(
                        out=_ap(
                            accd_ap, s * (G * M) * ACCF, [(ACCF, G * M), (1, ACCF)]
                        ),
                        in_=_ap(
                            accsb_ap, s * (G * M) * P_ASB, [(P_ASB, G * M), (1, ACCF)]
                        ),
                    )
                    for g in range(G):
                        pb = (s * G + g) * M
                        eng.dma_start(
                            out=_ap(
                                out2_ap,
                                (g * GC) * P_O2 + s * HW,
                                [(P_O2, GC), (RH * W, M), (1, RH * W)],
                            ),
                            in_=_ap(
                                accd_ap,
                                pb * ACCF,
                                [(RH * W, GC), (ACCF, M), (1, RH * W)],
                            ),
                        )



            # ---- conv3 + BN3 + residual + ReLU -> out (paired chunks) -----
            w3t_v = w3t_sb[:, :].rearrange("p (k c) -> p k c", k=2)
            with tc.tile_pool(name="ps3", bufs=4, space="PSUM") as p3:
                for s in range(S):
                    for oc in range(2):
                        for n0 in (0, 2, 4, 6):
                            npair = 1 if n0 == 6 else 2
                            ps = p3.tile([128, 1024], dt.float32, tag="ps3")
                            for j in range(npair):
                                n = n0 + j
                                psl = ps[:, j * 512 : j * 512 + NW]
                                nc.tensor.matmul(
                                    psl,
                                    w3t_v[:, oc, :],
                                    out2[:, s * HW + n * NW : s * HW + (n + 1) * NW],
                                    start=True,
                                    stop=False,
                                )
                                nc.tensor.matmul(
                                    psl,
                                    id_sb[:, :],
                                    xbf_v[:, s, oc, n * NW : (n + 1) * NW],
                                    start=False,
                                    stop=True,
                                )
                            ob = spool.tile([128, 2 * NW], dt.float32, tag="obuf")
                            nc.scalar.activation(
                                ob[:, : npair * NW],
                                _ap(
                                    ps[:, :],
                                    0,
                                    [(ps[:, :].ap[0][0], 128), (512, npair), (1, NW)],
                                ),
                                AF.Relu,
                                bias=b3_sb[:, oc : oc + 1],
                            )
                            nc.sync.dma_start(
                                out=out.ap()[
                                    s,
                                    oc * 128 : (oc + 1) * 128,
                                    n0 * NW : (n0 + npair) * NW,
                                ],
                                in_=ob[:, : npair * NW],
                            )

    nc.compile()
    _CACHE["nc"] = nc
    return nc


def _f32(a):
    return np.ascontiguousarray(a, dtype=np.float32)


def prep_weights(inputs):
    """Host-side folding of BN scales into conv weights; bf16 casts."""
    f = inputs
    s1 = f["bn1_g"] / np.sqrt(f["bn1_v"] + EPS)
    b1_eff = f["bn1_b"] - f["bn1_m"] * s1
    w1t_eff = (_f32(f["conv1_w"]) * s1[:, None]).T          # [256, 64]

    si = f["inv_bn_g"] / np.sqrt(f["inv_bn_v"] + EPS)
    bi_eff = f["inv_bn_b"] - f["inv_bn_m"] * si
    c1t_eff = (_f32(f["inv_c1_w"]) * si[:, None]).T         # [64, 16]

    c2t_eff = _f32(f["inv_c2_w"]).T                         # [16, 196]
    b2c = _f32(f["inv_c2_b"])

    # relu(s2*y + b2n) = s2 * relu(y + b2n/s2), valid because s2 > 0: the
    # scale folds into conv3's input columns, the shifted bias is added in
    # PSUM by the gsel/b2pat matmul, and the accumulator evac applies the relu
    s2 = _f32(f["bn2_g"] / np.sqrt(f["bn2_v"] + EPS))
    b2n = _f32(f["bn2_b"] - f["bn2_m"] * s2)
    s3 = f["bn3_g"] / np.sqrt(f["bn3_v"] + EPS)
    b3_eff = f["bn3_b"] - f["bn3_m"] * s3
    w3_eff = _f32(f["conv3_w"]) * s3[:, None] * s2[None, :]  # [256, 64]
    w3t_eff = w3_eff.T                                       # [64, 256]

    d = {}
    d["w1t"] = np.ascontiguousarray(
        w1t_eff.reshape(2, 128, CMID).astype(BF16)
    )
    d["b1"] = _f32(b1_eff)[:, None]
    d["c1t"] = np.ascontiguousarray(c1t_eff.astype(BF16))
    d["bi"] = _f32(bi_eff)[:, None]
    c2ta_x = np.concatenate([c2t_eff[:, 0:98], b2c[None, 0:98]], axis=0)
    c2tb_x = np.concatenate([c2t_eff[:, 98:196], b2c[None, 98:196]], axis=0)
    d["c2ta"] = np.ascontiguousarray(c2ta_x.astype(BF16))
    d["c2tb"] = np.ascontiguousarray(c2tb_x.astype(BF16))
    d["w3t"] = np.ascontiguousarray(
        w3t_eff.reshape(CMID, 2, 128).transpose(1, 0, 2).astype(BF16)
    )
    d["b3"] = _f32(b3_eff.reshape(2, 128).T)
    d["ident"] = np.ascontiguousarray(np.eye(128, dtype=np.float32).astype(BF16))
    b2r = b2n / s2           # relu-shifted BN2 bias per mid channel [64]
    d["b2pat"] = np.ascontiguousarray(
        np.repeat(b2r.reshape(G, GC), RH * W, axis=1).astype(BF16)
    )
    d["zone"] = np.ones((1, S * HW), dtype=BF16)
    gs = np.zeros((G, 128), np.float32)
    for p in range(NP_INV):
        gs[(p // M) % G, p] = 1.0
    d["gsel"] = np.ascontiguousarray(gs.astype(BF16))
    return d


def make_in_maps(inputs):
    prep = prep_weights(inputs)
    x = _f32(inputs["x"]).reshape(16, CIN, HW)
    in_maps = []
    for i in range(N_CORES):
        m = dict(prep)
        m["xin"] = np.ascontiguousarray(x[S * i : S * i + S])
        in_maps.append(m)
    return in_maps


def kernel(**inputs):
    from concourse.bass_utils import run_bass_kernel_spmd

    nc = build_module()
    in_maps = make_in_maps(inputs)
    res = run_bass_kernel_spmd(nc, in_maps, core_ids=list(range(N_CORES)))
    outs = [res.results[i]["out"].reshape(S, CIN, H, W) for i in range(N_CORES)]
    return np.concatenate(outs, axis=0).astype(np.float32)
